# revision 3
# baseline (speedup 1.0000x reference)
"""MixLoss Trainium2 kernel v2: trees on E, three-engine E production.

loss = 0.5*(ce + nll) over tokens, with
  ce  = -mean[ log_softmax_c(segment_max_f(logits))[label] ]
  nll = -mean[ log(S[label] / Z) ],  S_c = sum_{f in c} e^x_f, Z = sum_f e^x_f

Structural change vs v1: the segment-max trees run on E = exp(x) (positive
bf16; max commutes with exp) instead of on the raw logits, so the DMA dtype
no longer pins the tree dtype -> ALL logits ship as fp8 (half the HBM
traffic). E is produced by two engines in parallel, split by column range:
  - ACT: plain exp (dtype-agnostic, 0.855 ns/elem)
  - Pool: Schraudolph bit-trick exp via tensor_scalar (the only tensor op
    the Pool engine accepts): i16 = round(x*128/ln2 + B); bitcast(i16) as
    bf16 ~= e^x with ~4% sawtooth error, B tuned for ~zero mean bias
    (end-to-end loss err ~5e-5, tolerance is 2e-2).
Tree max over E yields EM = e^{segment max} directly (no separate exp of
the maxes). PE does the segmented sums over E -> S_c per class (psum), DVE
reduces S_c -> Z; PE also sums EM -> sum_em and the label-row sums.

Data-parallel over 8 cores (batch split); 8192 tokens/core = 64 tiles of
128 tokens (tokens on SBUF partitions).
"""

import ml_dtypes
import numpy as np

import concourse.bacc as bacc
import concourse.mybir as mybir
from concourse import tile
from concourse.bass_utils import run_bass_kernel_spmd

N_CORES = 8
P = 128                    # SBUF partitions = tokens per tile
SB = 8                     # PE/PSUM sub-block (one PSUM bank)
N_TILES = 64
SWEEPS = (8, 8, 16, 16, 16)  # tiles per tree sweep (must sum to 64)
ACT_FRAC = 0.66            # ACT's column share of E production (Pool rest)
E_BUFS = 3
SCR_BUFS = 2
LG_BUFS = 3
DVE_FILL_FRAC = 0.45       # DVE's E share in sweep 0 (pipeline fill)
PRELOAD_ACT_TABLE = None   # act_info.json set 6 = exp+ln; off (no sim delta)
PS_TILES = 8               # psum Z-granularity in tiles (16 = 2 banks)
PS_BUFS = 3
SCHR_A = float(128.0 / np.log(2.0))
SCHR_B = 16248.6

F32 = mybir.dt.float32
BF16 = mybir.dt.bfloat16
I16 = mybir.dt.int16
FP8 = mybir.dt.float8e4
AF = mybir.ActivationFunctionType
ALU = mybir.AluOpType
AX = mybir.AxisListType

_prog_cache = {}


def _tree_instrs(cap):
    # instruction count of _tier_tree's halving chain
    if cap == 2:
        return 1
    n, w = 1, cap // 2
    while w > 2:
        if w % 2 == 1:
            w -= 1
        else:
            w //= 2
        n += 1
    return n + 1


def _hybrid_plan(cap, n_el):
    """Pick how many halving levels to run on DVE before finishing with a
    single tensor_reduce, by modeled DVE cost (0.535 ns/el tree 2x mode,
    1.10 ns/el reduce, ~110 ns per instruction)."""
    best = (None, float("inf"))
    # k = number of halving instructions executed (following _tier_tree's
    # odd-fold chain); simulate the chain to cost each prefix
    w = cap
    done = 0.0
    k = 0
    cost_full_tree = 0.535 * n_el * (1.0 - 1.0 / cap) + 110.0 * _tree_instrs(cap)
    best = ("tree", cost_full_tree)
    while True:
        # cost of: k instrs so far (processing `done` fraction) + reduce tail
        if w >= 2:
            c = done + (1.10 * n_el * w / cap + 110.0)
            if c < best[1]:
                best = (k, c)
        if w == 2 or w == 1:
            break
        if w % 2 == 1:
            done += 0.535 * n_el / cap + 110.0
            w -= 1
        else:
            done += 0.535 * n_el * (w / 2) / cap + 110.0
            w //= 2
        k += 1
    return best[0]


def _tier_emit(nc, src4, scr4, dest, cap, SW, ncls):
    """Emit the segment max for one tier: either the full halving tree or
    k halving levels followed by one tensor_reduce over the remaining
    width, whichever the cost model prefers."""
    AXX = mybir.AxisListType.X
    n_el = SW * ncls * cap
    plan = _hybrid_plan(cap, n_el)
    if plan == "tree":
        _tier_tree(nc, src4, scr4, dest, cap)
        return
    k = plan
    if k == 0:
        nc.vector.tensor_reduce(dest, src4, axis=AXX, op=ALU.max)
        return
    v = nc.vector
    op = ALU.max
    half = cap // 2
    v.tensor_tensor(
        scr4[:, :, :, 0:half], src4[:, :, :, 0:half], src4[:, :, :, half:cap], op=op
    )
    w = half
    k -= 1
    while k > 0:
        if w % 2 == 1:
            v.tensor_tensor(
                scr4[:, :, :, 0:1], scr4[:, :, :, 0:1], scr4[:, :, :, w - 1 : w], op=op
            )
            w -= 1
        else:
            h = w // 2
            v.tensor_tensor(
                scr4[:, :, :, 0:h], scr4[:, :, :, 0:h], scr4[:, :, :, h:w], op=op
            )
            w = h
        k -= 1
    v.tensor_reduce(dest, scr4[:, :, :, 0:w], axis=AXX, op=op)


def _tier_tree(nc, src4, scr4, dest, cap):
    """Segment max over the last axis (width cap, even) of src4 [p,t,c,cap]
    into dest [p,t,c] via pairwise halving on DVE (2x bf16 mode). Odd
    intermediate widths fold the straggler into slot 0."""
    op = ALU.max
    v = nc.vector
    assert cap % 2 == 0
    if cap == 2:
        v.tensor_tensor(dest, src4[:, :, :, 0:1], src4[:, :, :, 1:2], op=op)
        return
    half = cap // 2
    v.tensor_tensor(
        scr4[:, :, :, 0:half], src4[:, :, :, 0:half], src4[:, :, :, half:cap], op=op
    )
    w = half
    while True:
        if w == 2:
            v.tensor_tensor(dest, scr4[:, :, :, 0:1], scr4[:, :, :, 1:2], op=op)
            return
        if w % 2 == 1:
            v.tensor_tensor(
                scr4[:, :, :, 0:1], scr4[:, :, :, 0:1], scr4[:, :, :, w - 1 : w], op=op
            )
            w -= 1
        else:
            h = w // 2
            v.tensor_tensor(
                scr4[:, :, :, 0:h], scr4[:, :, :, 0:h], scr4[:, :, :, h:w], op=op
            )
            w = h


def _build_program(n_tiles: int, NIDX: int, C: int, tiers: tuple, capmax: int):
    # tiers: ((cap, c0, c1, off), ...) with off = slot offset of the tier.
    nc = bacc.Bacc()

    logits8_d = nc.dram_tensor("logits8", [P, n_tiles, NIDX], FP8, kind="ExternalInput")
    lab_d = nc.dram_tensor("labrows", [P, n_tiles, capmax], BF16, kind="ExternalInput")
    eye_d = nc.dram_tensor("eye", [P, P], BF16, kind="ExternalInput")
    out_d = nc.dram_tensor("out", [P, 1], F32, kind="ExternalOutput")

    assert sum(SWEEPS) == n_tiles and all(s % SB == 0 for s in SWEEPS)
    n_sweeps = len(SWEEPS)
    act_cols = int(round(NIDX * ACT_FRAC))

    with tile.TileContext(nc) as tc:
        with (
            tc.tile_pool(name="const", bufs=1) as cpool,
            tc.tile_pool(name="blk", bufs=1) as bpool,
            tc.psum_pool(name="ps", bufs=1) as ppool,
        ):
            eye = cpool.tile([P, P], BF16)
            if PRELOAD_ACT_TABLE is not None:
                _ld = mybir.InstLoadActFuncSet(
                    name=nc.get_next_instruction_name(), ins=[], outs=[],
                    act_func_set_id=PRELOAD_ACT_TABLE,
                )
                _ld.engine = mybir.EngineType.Activation
                nc.scalar.add_instruction(_ld)
            nc.sync.dma_start(eye[:, :], eye_d[:, :])
            em_all = cpool.tile([P, n_tiles * C], BF16)
            # packed [num | den] so the final Ln is one instruction
            nd = cpool.tile([P, 2 * n_tiles], F32)
            zt = cpool.tile([P, n_tiles], F32)

            def lab_path():
                # label-row path: num = EM[label] * S[label] per token
                lab = cpool.tile([P, n_tiles * capmax], BF16)
                nc.sync.dma_start(lab[:, :], lab_d.rearrange("p t g -> p (t g)"))
                nc.scalar.activation(lab[:, :], lab[:, :], AF.Exp)
                lab3 = lab.rearrange("p (t g) -> p t g", g=capmax)
                em_l = cpool.tile([P, n_tiles], BF16)
                nc.vector.tensor_reduce(em_l[:, :], lab3, axis=AX.X, op=ALU.max)
                # S[label] on PE: psum[p,t] += lab3[p,t,j]
                psl = ppool.tile([P, n_tiles], F32, tag="psl", bufs=1)
                for j in range(capmax):
                    nc.tensor.matmul(
                        psl[:, :],
                        eye[:, :],
                        lab3[:, :, j : j + 1],
                        start=(j == 0),
                        stop=(j == capmax - 1),
                    )
                with nc.allow_low_precision("bf16 em_l; noise averages out"):
                    nc.vector.tensor_mul(nd[:, 0:n_tiles], em_l[:, :], psl[:, :])

            with nc.allow_low_precision("schraudolph E; noise averages out"):
                pend_z = []
                ps_cur, ps_t0, ps_fill = None, 0, 0
                sw_starts = [sum(SWEEPS[:i]) for i in range(n_sweeps)]
                SWMAX = max(SWEEPS)
                for sw_i in range(n_sweeps):
                    sw_t0 = sw_starts[sw_i]
                    SW = SWEEPS[sw_i]
                    # E for this sweep (bf16; Pool writes via int16 bitcast)
                    e_full = bpool.tile([P, SWMAX * NIDX], BF16, tag="e", bufs=E_BUFS)
                    e_sw = e_full[:, : SW * NIDX]
                    for s_i in range(SW // SB):
                        t0s = sw_t0 + s_i * SB
                        e_sub = e_sw[:, s_i * SB * NIDX : (s_i + 1) * SB * NIDX]
                        lg = bpool.tile([P, SB * NIDX], FP8, tag="lg8", bufs=LG_BUFS)
                        # per-half-subblock DMA so each E op waits only on
                        # its own chunk
                        for h in range(2):
                            h0 = h * (SB // 2)
                            nc.sync.dma_start(
                                lg[:, h0 * NIDX : (h0 + SB // 2) * NIDX],
                                logits8_d[:, t0s + h0 : t0s + h0 + SB // 2, :],
                            )
                            lg3 = lg.rearrange("p (t i) -> p t i", i=NIDX)
                            e3 = e_sub.rearrange("p (t i) -> p t i", i=NIDX)
                            tl = slice(h0, h0 + SB // 2)
                            if sw_i == 0 and DVE_FILL_FRAC > 0:
                                # pipeline fill: DVE shares sweep-0 E so its
                                # trees start sooner
                                a_c = int(round(NIDX * ACT_FRAC * (1 - DVE_FILL_FRAC)))
                                d_c = a_c + int(round(NIDX * DVE_FILL_FRAC))
                                nc.scalar.activation(
                                    e3[:, tl, 0:a_c], lg3[:, tl, 0:a_c], AF.Exp,
                                )
                                nc.vector.tensor_scalar(
                                    e3[:, tl, a_c:d_c].bitcast(I16),
                                    lg3[:, tl, a_c:d_c],
                                    scalar1=SCHR_A, scalar2=SCHR_B,
                                    op0=ALU.mult, op1=ALU.add,
                                )
                                nc.gpsimd.tensor_scalar(
                                    e3[:, tl, d_c:NIDX].bitcast(I16),
                                    lg3[:, tl, d_c:NIDX],
                                    scalar1=SCHR_A, scalar2=SCHR_B,
                                    op0=ALU.mult, op1=ALU.add,
                                )
                            else:
                                # ACT takes act_cols per tile, Pool the rest
                                nc.scalar.activation(
                                    e3[:, tl, 0:act_cols], lg3[:, tl, 0:act_cols],
                                    AF.Exp,
                                )
                                nc.gpsimd.tensor_scalar(
                                    e3[:, tl, act_cols:NIDX].bitcast(I16),
                                    lg3[:, tl, act_cols:NIDX],
                                    scalar1=SCHR_A, scalar2=SCHR_B,
                                    op0=ALU.mult, op1=ALU.add,
                                )
                        # PE segmented sums for this subblock -> S_c; Z on DVE
                        es3 = e_sub.rearrange("p (t i) -> p t i", i=NIDX)
                        if ps_cur is None:
                            ps_w = min(PS_TILES, SW - s_i * SB)
                            ps_tile = ppool.tile(
                                [P, ps_w * C], F32, tag="ps", bufs=PS_BUFS
                            )
                            ps_cur = ps_tile.rearrange("p (t c) -> p t c", c=C)
                            ps_t0, ps_fill = t0s, 0
                        ps3 = ps_cur[:, ps_fill : ps_fill + SB, :]
                        for (cap, c0, c1, off) in tiers:
                            ncls = c1 - c0
                            src4 = es3[:, :, off : off + ncls * cap].rearrange(
                                "p t (c g) -> p t c g", g=cap
                            )
                            for j in range(cap):
                                nc.tensor.matmul(
                                    ps3[:, :, c0:c1],
                                    eye[:, :],
                                    src4[:, :, :, j : j + 1],
                                    start=(j == 0),
                                    stop=(j == cap - 1),
                                )
                        ps_fill += SB
                        if ps_fill == ps_cur.shape[1]:
                            pend_z.append((ps_t0, ps_cur))
                            ps_cur = None

                    # tree sweep: EM = segment max of E (DVE halving)
                    es3 = e_sw.rearrange("p (t i) -> p t i", i=NIDX)
                    scr_full = bpool.tile(
                        [P, SWMAX * (NIDX // 2)], BF16, tag="scr", bufs=SCR_BUFS
                    )
                    ss3 = scr_full[:, : SW * (NIDX // 2)].rearrange(
                        "p (t i) -> p t i", i=NIDX // 2
                    )
                    em_b = em_all[
                        :, sw_t0 * C : (sw_t0 + SW) * C
                    ].rearrange("p (t c) -> p t c", c=C)
                    last = sw_i == n_sweeps - 1
                    for (cap, c0, c1, off) in tiers:
                        ncls = c1 - c0
                        src4 = es3[:, :, off : off + ncls * cap].rearrange(
                            "p t (c g) -> p t c g", g=cap
                        )
                        scr4 = ss3[
                            :, :, off // 2 : off // 2 + ncls * (cap // 2)
                        ].rearrange("p t (c g) -> p t c g", g=cap // 2)
                        if last:
                            with tc.high_priority():
                                _tier_emit(nc, src4, scr4, em_b[:, :, c0:c1],
                                           cap, SW, ncls)
                        else:
                            _tier_emit(nc, src4, scr4, em_b[:, :, c0:c1],
                                       cap, SW, ncls)
                    for (t0s, ps3_) in pend_z:
                        nc.vector.tensor_reduce(
                            zt[:, t0s : t0s + ps3_.shape[1]], ps3_,
                            axis=AX.X, op=ALU.add,
                        )
                    pend_z = []
                    # sum_em = sum_c EM on PE; den = sum_em * Z
                    pse = ppool.tile([P, SWMAX], F32, tag="pse", bufs=2)
                    pse = pse[:, :SW]
                    for c in range(C):
                        nc.tensor.matmul(
                            pse[:, :],
                            eye[:, :],
                            em_b[:, :, c : c + 1],
                            start=(c == 0),
                            stop=(c == C - 1),
                        )
                    nc.vector.tensor_mul(
                        nd[:, n_tiles + sw_t0 : n_tiles + sw_t0 + SW],
                        pse[:, :],
                        zt[:, sw_t0 : sw_t0 + SW],
                    )
                    if sw_i == 0:
                        # label-row path lands mid-stream: its DMA/exp fits
                        # engine gaps without delaying fill or tail
                        lab_path()

            nt_head = n_tiles - SWEEPS[-1]
            lnd = cpool.tile([P, 2 * n_tiles], F32)
            nd3h = nd.rearrange("p (s t) -> p s t", s=2)[:, :, 0:nt_head]
            lnd3h = lnd.rearrange("p (s t) -> p s t", s=2)[:, :, 0:nt_head]
            nc.scalar.activation(lnd3h, nd3h, AF.Ln)
            term = cpool.tile([P, n_tiles], F32)
            nc.vector.tensor_sub(
                term[:, 0:nt_head], lnd[:, 0:nt_head],
                lnd[:, n_tiles : n_tiles + nt_head],
            )
            with tc.high_priority():
                nd3t = nd.rearrange("p (s t) -> p s t", s=2)[:, :, nt_head:n_tiles]
                lnd3t = lnd.rearrange("p (s t) -> p s t", s=2)[:, :, nt_head:n_tiles]
                nc.scalar.activation(lnd3t, nd3t, AF.Ln)
                nc.vector.tensor_sub(
                    term[:, nt_head:n_tiles], lnd[:, nt_head:n_tiles],
                    lnd[:, n_tiles + nt_head : 2 * n_tiles],
                )
                acc = cpool.tile([P, 1], F32)
                nc.vector.tensor_reduce(acc[:, :], term[:, :], axis=AX.X, op=ALU.add)
                nc.sync.dma_start(out_d[:, :], acc[:, :])

    nc.finalize()
    return nc


def _prepare(logits, labels, mask_matrix):
    Bb, S, F = logits.shape
    C = mask_matrix.shape[1]
    n_tok = Bb * S
    tok_per_core = n_tok // N_CORES
    n_tiles = tok_per_core // P

    seg = np.asarray(mask_matrix).argmax(axis=1)
    members0 = [np.nonzero(seg == c)[0] for c in range(C)]
    sizes = np.array([len(m) for m in members0])
    caps = np.maximum(2, -(-sizes // 2) * 2)  # even capacities
    perm = np.argsort(caps, kind="stable")
    members = [members0[c] for c in perm]
    caps = caps[perm].astype(np.int64)
    tier_list = []
    offs = np.concatenate([[0], np.cumsum(caps)])
    NIDX = int(offs[-1])
    c0 = 0
    for c in range(1, C + 1):
        if c == C or caps[c] != caps[c0]:
            tier_list.append((int(caps[c0]), c0, c, int(offs[c0])))
            c0 = c
    tiers = tuple(tier_list)
    capmax = int(caps.max())

    # source fine-index per slot; pads -> appended -20 column (E=0)
    src_idx = np.full(NIDX, F, dtype=np.int64)
    for c, m in enumerate(members):
        src_idx[offs[c] : offs[c] + len(m)] = m

    lf = np.asarray(logits, dtype=np.float32).reshape(n_tok, F)
    lf = np.concatenate([lf, np.full((n_tok, 1), -20.0, dtype=np.float32)], axis=1)
    lb = lf.astype(ml_dtypes.bfloat16)
    lg32 = lf[:, src_idx]  # [n_tok, NIDX] grouped+padded, fp32

    inv_perm = np.empty(C, dtype=np.int64)
    inv_perm[perm] = np.arange(C)
    lab = inv_perm[np.asarray(labels).reshape(-1).astype(np.int64)]
    j = np.arange(capmax)[None, :]
    col_f = np.where(
        j < caps[lab][:, None],
        src_idx[np.minimum(offs[lab][:, None] + j, NIDX - 1)],
        F,
    )
    lab_rows = np.take_along_axis(lb, col_f, axis=1)

    lg8 = np.ascontiguousarray(
        lg32.reshape(N_CORES, n_tiles, P, NIDX).transpose(0, 2, 1, 3)
    ).astype(ml_dtypes.float8_e4m3fn)
    lab_rows = np.ascontiguousarray(
        lab_rows.reshape(N_CORES, n_tiles, P, capmax).transpose(0, 2, 1, 3)
    )
    eye = np.eye(P, dtype=ml_dtypes.bfloat16)
    return lg8, lab_rows, eye, tiers, n_tiles, NIDX, C, capmax, n_tok


def _run(logits, labels, mask_matrix, **spmd_kwargs):
    lg8, lab_rows, eye, tiers, n_tiles, NIDX, C, capmax, n_tok = _prepare(
        logits, labels, mask_matrix
    )
    key = (n_tiles, NIDX, C, tiers, capmax)
    if key not in _prog_cache:
        _prog_cache[key] = _build_program(*key)
    nc = _prog_cache[key]
    in_maps = [
        {"logits8": lg8[k], "labrows": lab_rows[k], "eye": eye}
        for k in range(N_CORES)
    ]
    res = run_bass_kernel_spmd(nc, in_maps, core_ids=list(range(N_CORES)), **spmd_kwargs)
    total = np.float64(0.0)
    for r in res.results:
        total += np.float64(r["out"].sum(dtype=np.float64))
    loss = np.float32(-0.5 * total / n_tok)
    return loss, res


def kernel(logits, labels, mask_matrix):
    loss, _ = _run(logits, labels, mask_matrix)
    return loss


# revision 4
# speedup vs baseline: 1.0149x; 1.0149x over previous
"""MixLoss Trainium2 kernel v2: trees on E, three-engine E production.

loss = 0.5*(ce + nll) over tokens, with
  ce  = -mean[ log_softmax_c(segment_max_f(logits))[label] ]
  nll = -mean[ log(S[label] / Z) ],  S_c = sum_{f in c} e^x_f, Z = sum_f e^x_f

Structural change vs v1: the segment-max trees run on E = exp(x) (positive
bf16; max commutes with exp) instead of on the raw logits, so the DMA dtype
no longer pins the tree dtype -> ALL logits ship as fp8 (half the HBM
traffic). E is produced by two engines in parallel, split by column range:
  - ACT: plain exp (dtype-agnostic, 0.855 ns/elem)
  - Pool: Schraudolph bit-trick exp via tensor_scalar (the only tensor op
    the Pool engine accepts): i16 = round(x*128/ln2 + B); bitcast(i16) as
    bf16 ~= e^x with ~4% sawtooth error, B tuned for ~zero mean bias
    (end-to-end loss err ~5e-5, tolerance is 2e-2).
Tree max over E yields EM = e^{segment max} directly (no separate exp of
the maxes). PE does the segmented sums over E -> S_c per class (psum), DVE
reduces S_c -> Z; PE also sums EM -> sum_em and the label-row sums.

Data-parallel over 8 cores (batch split); 8192 tokens/core = 64 tiles of
128 tokens (tokens on SBUF partitions).
"""

import ml_dtypes
import numpy as np

import concourse.bacc as bacc
import concourse.mybir as mybir
from concourse import tile
from concourse.bass_utils import run_bass_kernel_spmd

N_CORES = 8
P = 128                    # SBUF partitions = tokens per tile
SB = 8                     # PE/PSUM sub-block (one PSUM bank)
N_TILES = 64
SWEEPS = (8, 8, 16, 16, 16)  # tiles per tree sweep (must sum to 64)
ACT_FRAC = 0.66            # ACT's column share of E production (Pool rest)
E_BUFS = 3
SCR_BUFS = 2
LG_BUFS = 3
DVE_FILL_FRAC = 0.45       # DVE's E share in fill sweeps (pipeline fill)
FILL_SWEEPS = 1            # how many leading sweeps DVE helps with E
PRELOAD_ACT_TABLE = None   # act_info.json set 6 = exp+ln; off (no sim delta)
PS_TILES = 8               # psum Z-granularity in tiles (16 = 2 banks)
PS_BUFS = 3
SCHR_A = float(128.0 / np.log(2.0))
SCHR_B = 16248.6

F32 = mybir.dt.float32
BF16 = mybir.dt.bfloat16
I16 = mybir.dt.int16
FP8 = mybir.dt.float8e4
AF = mybir.ActivationFunctionType
ALU = mybir.AluOpType
AX = mybir.AxisListType

_prog_cache = {}


def _tree_instrs(cap):
    # instruction count of _tier_tree's halving chain
    if cap == 2:
        return 1
    n, w = 1, cap // 2
    while w > 2:
        if w % 2 == 1:
            w -= 1
        else:
            w //= 2
        n += 1
    return n + 1


def _hybrid_plan(cap, n_el):
    """Pick how many halving levels to run on DVE before finishing with a
    single tensor_reduce, by modeled DVE cost (0.535 ns/el tree 2x mode,
    1.10 ns/el reduce, ~110 ns per instruction)."""
    best = (None, float("inf"))
    # k = number of halving instructions executed (following _tier_tree's
    # odd-fold chain); simulate the chain to cost each prefix
    w = cap
    done = 0.0
    k = 0
    cost_full_tree = 0.535 * n_el * (1.0 - 1.0 / cap) + 110.0 * _tree_instrs(cap)
    best = ("tree", cost_full_tree)
    while True:
        # cost of: k instrs so far (processing `done` fraction) + reduce tail
        if w >= 2:
            c = done + (1.10 * n_el * w / cap + 110.0)
            if c < best[1]:
                best = (k, c)
        if w == 2 or w == 1:
            break
        if w % 2 == 1:
            done += 0.535 * n_el / cap + 110.0
            w -= 1
        else:
            done += 0.535 * n_el * (w / 2) / cap + 110.0
            w //= 2
        k += 1
    return best[0]


def _tier_emit(nc, src4, scr4, dest, cap, SW, ncls):
    """Emit the segment max for one tier: either the full halving tree or
    k halving levels followed by one tensor_reduce over the remaining
    width, whichever the cost model prefers."""
    AXX = mybir.AxisListType.X
    n_el = SW * ncls * cap
    plan = _hybrid_plan(cap, n_el)
    if plan == "tree":
        _tier_tree(nc, src4, scr4, dest, cap)
        return
    k = plan
    if k == 0:
        nc.vector.tensor_reduce(dest, src4, axis=AXX, op=ALU.max)
        return
    v = nc.vector
    op = ALU.max
    half = cap // 2
    v.tensor_tensor(
        scr4[:, :, :, 0:half], src4[:, :, :, 0:half], src4[:, :, :, half:cap], op=op
    )
    w = half
    k -= 1
    while k > 0:
        if w % 2 == 1:
            v.tensor_tensor(
                scr4[:, :, :, 0:1], scr4[:, :, :, 0:1], scr4[:, :, :, w - 1 : w], op=op
            )
            w -= 1
        else:
            h = w // 2
            v.tensor_tensor(
                scr4[:, :, :, 0:h], scr4[:, :, :, 0:h], scr4[:, :, :, h:w], op=op
            )
            w = h
        k -= 1
    v.tensor_reduce(dest, scr4[:, :, :, 0:w], axis=AXX, op=op)


def _tier_tree(nc, src4, scr4, dest, cap):
    """Segment max over the last axis (width cap, even) of src4 [p,t,c,cap]
    into dest [p,t,c] via pairwise halving on DVE (2x bf16 mode). Odd
    intermediate widths fold the straggler into slot 0."""
    op = ALU.max
    v = nc.vector
    assert cap % 2 == 0
    if cap == 2:
        v.tensor_tensor(dest, src4[:, :, :, 0:1], src4[:, :, :, 1:2], op=op)
        return
    half = cap // 2
    v.tensor_tensor(
        scr4[:, :, :, 0:half], src4[:, :, :, 0:half], src4[:, :, :, half:cap], op=op
    )
    w = half
    while True:
        if w == 2:
            v.tensor_tensor(dest, scr4[:, :, :, 0:1], scr4[:, :, :, 1:2], op=op)
            return
        if w % 2 == 1:
            v.tensor_tensor(
                scr4[:, :, :, 0:1], scr4[:, :, :, 0:1], scr4[:, :, :, w - 1 : w], op=op
            )
            w -= 1
        else:
            h = w // 2
            v.tensor_tensor(
                scr4[:, :, :, 0:h], scr4[:, :, :, 0:h], scr4[:, :, :, h:w], op=op
            )
            w = h


def _build_program(n_tiles: int, NIDX: int, C: int, tiers: tuple, capmax: int):
    # tiers: ((cap, c0, c1, off), ...) with off = slot offset of the tier.
    nc = bacc.Bacc()

    logits8_d = nc.dram_tensor("logits8", [P, n_tiles, NIDX], FP8, kind="ExternalInput")
    lab_d = nc.dram_tensor("labrows", [P, n_tiles, capmax], BF16, kind="ExternalInput")
    eye_d = nc.dram_tensor("eye", [P, P], BF16, kind="ExternalInput")
    out_d = nc.dram_tensor("out", [P, 1], F32, kind="ExternalOutput")

    assert sum(SWEEPS) == n_tiles and all(s % SB == 0 for s in SWEEPS)
    n_sweeps = len(SWEEPS)
    act_cols = int(round(NIDX * ACT_FRAC))

    with tile.TileContext(nc) as tc:
        with (
            tc.tile_pool(name="const", bufs=1) as cpool,
            tc.tile_pool(name="blk", bufs=1) as bpool,
            tc.psum_pool(name="ps", bufs=1) as ppool,
        ):
            eye = cpool.tile([P, P], BF16)
            if PRELOAD_ACT_TABLE is not None:
                _ld = mybir.InstLoadActFuncSet(
                    name=nc.get_next_instruction_name(), ins=[], outs=[],
                    act_func_set_id=PRELOAD_ACT_TABLE,
                )
                _ld.engine = mybir.EngineType.Activation
                nc.scalar.add_instruction(_ld)
            nc.sync.dma_start(eye[:, :], eye_d[:, :])
            em_all = cpool.tile([P, n_tiles * C], BF16)
            # packed [num | den] so the final Ln is one instruction
            nd = cpool.tile([P, 2 * n_tiles], F32)
            zt = cpool.tile([P, n_tiles], F32)

            def lab_path():
                # label-row path: num = EM[label] * S[label] per token
                lab = cpool.tile([P, n_tiles * capmax], BF16)
                nc.sync.dma_start(lab[:, :], lab_d.rearrange("p t g -> p (t g)"))
                nc.scalar.activation(lab[:, :], lab[:, :], AF.Exp)
                lab3 = lab.rearrange("p (t g) -> p t g", g=capmax)
                em_l = cpool.tile([P, n_tiles], BF16)
                nc.vector.tensor_reduce(em_l[:, :], lab3, axis=AX.X, op=ALU.max)
                # S[label] on PE: psum[p,t] += lab3[p,t,j]
                psl = ppool.tile([P, n_tiles], F32, tag="psl", bufs=1)
                for j in range(capmax):
                    nc.tensor.matmul(
                        psl[:, :],
                        eye[:, :],
                        lab3[:, :, j : j + 1],
                        start=(j == 0),
                        stop=(j == capmax - 1),
                    )
                with nc.allow_low_precision("bf16 em_l; noise averages out"):
                    nc.vector.tensor_mul(nd[:, 0:n_tiles], em_l[:, :], psl[:, :])

            with nc.allow_low_precision("schraudolph E; noise averages out"):
                pend_z = []
                ps_cur, ps_t0, ps_fill = None, 0, 0
                sw_starts = [sum(SWEEPS[:i]) for i in range(n_sweeps)]
                SWMAX = max(SWEEPS)
                for sw_i in range(n_sweeps):
                    sw_t0 = sw_starts[sw_i]
                    SW = SWEEPS[sw_i]
                    # E for this sweep (bf16; Pool writes via int16 bitcast)
                    e_full = bpool.tile([P, SWMAX * NIDX], BF16, tag="e", bufs=E_BUFS)
                    e_sw = e_full[:, : SW * NIDX]
                    for s_i in range(SW // SB):
                        t0s = sw_t0 + s_i * SB
                        e_sub = e_sw[:, s_i * SB * NIDX : (s_i + 1) * SB * NIDX]
                        lg = bpool.tile([P, SB * NIDX], FP8, tag="lg8", bufs=LG_BUFS)
                        # per-half-subblock DMA so each E op waits only on
                        # its own chunk
                        for h in range(2):
                            h0 = h * (SB // 2)
                            nc.sync.dma_start(
                                lg[:, h0 * NIDX : (h0 + SB // 2) * NIDX],
                                logits8_d[:, t0s + h0 : t0s + h0 + SB // 2, :],
                            )
                            lg3 = lg.rearrange("p (t i) -> p t i", i=NIDX)
                            e3 = e_sub.rearrange("p (t i) -> p t i", i=NIDX)
                            tl = slice(h0, h0 + SB // 2)
                            if sw_i < FILL_SWEEPS and DVE_FILL_FRAC > 0:
                                # pipeline fill: DVE shares sweep-0 E so its
                                # trees start sooner
                                a_c = int(round(NIDX * ACT_FRAC * (1 - DVE_FILL_FRAC)))
                                d_c = a_c + int(round(NIDX * DVE_FILL_FRAC))
                                nc.scalar.activation(
                                    e3[:, tl, 0:a_c], lg3[:, tl, 0:a_c], AF.Exp,
                                )
                                nc.vector.tensor_scalar(
                                    e3[:, tl, a_c:d_c].bitcast(I16),
                                    lg3[:, tl, a_c:d_c],
                                    scalar1=SCHR_A, scalar2=SCHR_B,
                                    op0=ALU.mult, op1=ALU.add,
                                )
                                nc.gpsimd.tensor_scalar(
                                    e3[:, tl, d_c:NIDX].bitcast(I16),
                                    lg3[:, tl, d_c:NIDX],
                                    scalar1=SCHR_A, scalar2=SCHR_B,
                                    op0=ALU.mult, op1=ALU.add,
                                )
                            else:
                                # ACT takes act_cols per tile, Pool the rest
                                nc.scalar.activation(
                                    e3[:, tl, 0:act_cols], lg3[:, tl, 0:act_cols],
                                    AF.Exp,
                                )
                                nc.gpsimd.tensor_scalar(
                                    e3[:, tl, act_cols:NIDX].bitcast(I16),
                                    lg3[:, tl, act_cols:NIDX],
                                    scalar1=SCHR_A, scalar2=SCHR_B,
                                    op0=ALU.mult, op1=ALU.add,
                                )
                        # PE segmented sums for this subblock -> S_c; Z on DVE
                        es3 = e_sub.rearrange("p (t i) -> p t i", i=NIDX)
                        if ps_cur is None:
                            ps_w = min(PS_TILES, SW - s_i * SB)
                            ps_tile = ppool.tile(
                                [P, ps_w * C], F32, tag="ps", bufs=PS_BUFS
                            )
                            ps_cur = ps_tile.rearrange("p (t c) -> p t c", c=C)
                            ps_t0, ps_fill = t0s, 0
                        ps3 = ps_cur[:, ps_fill : ps_fill + SB, :]
                        for (cap, c0, c1, off) in tiers:
                            ncls = c1 - c0
                            src4 = es3[:, :, off : off + ncls * cap].rearrange(
                                "p t (c g) -> p t c g", g=cap
                            )
                            for j in range(cap):
                                nc.tensor.matmul(
                                    ps3[:, :, c0:c1],
                                    eye[:, :],
                                    src4[:, :, :, j : j + 1],
                                    start=(j == 0),
                                    stop=(j == cap - 1),
                                )
                        ps_fill += SB
                        if ps_fill == ps_cur.shape[1]:
                            pend_z.append((ps_t0, ps_cur))
                            ps_cur = None

                    # tree sweep: EM = segment max of E (DVE halving)
                    es3 = e_sw.rearrange("p (t i) -> p t i", i=NIDX)
                    scr_full = bpool.tile(
                        [P, SWMAX * (NIDX // 2)], BF16, tag="scr", bufs=SCR_BUFS
                    )
                    ss3 = scr_full[:, : SW * (NIDX // 2)].rearrange(
                        "p (t i) -> p t i", i=NIDX // 2
                    )
                    em_b = em_all[
                        :, sw_t0 * C : (sw_t0 + SW) * C
                    ].rearrange("p (t c) -> p t c", c=C)
                    last = sw_i == n_sweeps - 1
                    for (cap, c0, c1, off) in tiers:
                        ncls = c1 - c0
                        src4 = es3[:, :, off : off + ncls * cap].rearrange(
                            "p t (c g) -> p t c g", g=cap
                        )
                        scr4 = ss3[
                            :, :, off // 2 : off // 2 + ncls * (cap // 2)
                        ].rearrange("p t (c g) -> p t c g", g=cap // 2)
                        if last:
                            with tc.high_priority():
                                _tier_emit(nc, src4, scr4, em_b[:, :, c0:c1],
                                           cap, SW, ncls)
                        else:
                            _tier_emit(nc, src4, scr4, em_b[:, :, c0:c1],
                                       cap, SW, ncls)
                    for (t0s, ps3_) in pend_z:
                        nc.vector.tensor_reduce(
                            zt[:, t0s : t0s + ps3_.shape[1]], ps3_,
                            axis=AX.X, op=ALU.add,
                        )
                    pend_z = []
                    # sum_em = sum_c EM on PE; den = sum_em * Z
                    pse = ppool.tile([P, SWMAX], F32, tag="pse", bufs=2)
                    pse = pse[:, :SW]
                    for c in range(C):
                        nc.tensor.matmul(
                            pse[:, :],
                            eye[:, :],
                            em_b[:, :, c : c + 1],
                            start=(c == 0),
                            stop=(c == C - 1),
                        )
                    nc.vector.tensor_mul(
                        nd[:, n_tiles + sw_t0 : n_tiles + sw_t0 + SW],
                        pse[:, :],
                        zt[:, sw_t0 : sw_t0 + SW],
                    )
                    if sw_i == 0:
                        # label-row path lands mid-stream: its DMA/exp fits
                        # engine gaps without delaying fill or tail
                        lab_path()

            nt_head = n_tiles - SWEEPS[-1]
            lnd = cpool.tile([P, 2 * n_tiles], F32)
            nd3h = nd.rearrange("p (s t) -> p s t", s=2)[:, :, 0:nt_head]
            lnd3h = lnd.rearrange("p (s t) -> p s t", s=2)[:, :, 0:nt_head]
            nc.scalar.activation(lnd3h, nd3h, AF.Ln)
            term = cpool.tile([P, n_tiles], F32)
            nc.vector.tensor_sub(
                term[:, 0:nt_head], lnd[:, 0:nt_head],
                lnd[:, n_tiles : n_tiles + nt_head],
            )
            with tc.high_priority():
                nd3t = nd.rearrange("p (s t) -> p s t", s=2)[:, :, nt_head:n_tiles]
                lnd3t = lnd.rearrange("p (s t) -> p s t", s=2)[:, :, nt_head:n_tiles]
                nc.scalar.activation(lnd3t, nd3t, AF.Ln)
                nc.vector.tensor_sub(
                    term[:, nt_head:n_tiles], lnd[:, nt_head:n_tiles],
                    lnd[:, n_tiles + nt_head : 2 * n_tiles],
                )
                acc = cpool.tile([P, 1], F32)
                nc.vector.tensor_reduce(acc[:, :], term[:, :], axis=AX.X, op=ALU.add)
                nc.sync.dma_start(out_d[:, :], acc[:, :])

    nc.finalize()
    return nc


def _prepare(logits, labels, mask_matrix):
    Bb, S, F = logits.shape
    C = mask_matrix.shape[1]
    n_tok = Bb * S
    tok_per_core = n_tok // N_CORES
    n_tiles = tok_per_core // P

    seg = np.asarray(mask_matrix).argmax(axis=1)
    members0 = [np.nonzero(seg == c)[0] for c in range(C)]
    sizes = np.array([len(m) for m in members0])
    caps = np.maximum(2, -(-sizes // 2) * 2)  # even capacities
    perm = np.argsort(caps, kind="stable")
    members = [members0[c] for c in perm]
    caps = caps[perm].astype(np.int64)
    tier_list = []
    offs = np.concatenate([[0], np.cumsum(caps)])
    NIDX = int(offs[-1])
    c0 = 0
    for c in range(1, C + 1):
        if c == C or caps[c] != caps[c0]:
            tier_list.append((int(caps[c0]), c0, c, int(offs[c0])))
            c0 = c
    tiers = tuple(tier_list)
    capmax = int(caps.max())

    # source fine-index per slot; pads -> appended -20 column (E=0)
    src_idx = np.full(NIDX, F, dtype=np.int64)
    for c, m in enumerate(members):
        src_idx[offs[c] : offs[c] + len(m)] = m

    lf = np.asarray(logits, dtype=np.float32).reshape(n_tok, F)
    lf = np.concatenate([lf, np.full((n_tok, 1), -20.0, dtype=np.float32)], axis=1)
    lb = lf.astype(ml_dtypes.bfloat16)
    lg32 = lf[:, src_idx]  # [n_tok, NIDX] grouped+padded, fp32

    inv_perm = np.empty(C, dtype=np.int64)
    inv_perm[perm] = np.arange(C)
    lab = inv_perm[np.asarray(labels).reshape(-1).astype(np.int64)]
    j = np.arange(capmax)[None, :]
    col_f = np.where(
        j < caps[lab][:, None],
        src_idx[np.minimum(offs[lab][:, None] + j, NIDX - 1)],
        F,
    )
    lab_rows = np.take_along_axis(lb, col_f, axis=1)

    lg8 = np.ascontiguousarray(
        lg32.reshape(N_CORES, n_tiles, P, NIDX).transpose(0, 2, 1, 3)
    ).astype(ml_dtypes.float8_e4m3fn)
    lab_rows = np.ascontiguousarray(
        lab_rows.reshape(N_CORES, n_tiles, P, capmax).transpose(0, 2, 1, 3)
    )
    eye = np.eye(P, dtype=ml_dtypes.bfloat16)
    return lg8, lab_rows, eye, tiers, n_tiles, NIDX, C, capmax, n_tok


def _run(logits, labels, mask_matrix, **spmd_kwargs):
    lg8, lab_rows, eye, tiers, n_tiles, NIDX, C, capmax, n_tok = _prepare(
        logits, labels, mask_matrix
    )
    key = (n_tiles, NIDX, C, tiers, capmax)
    if key not in _prog_cache:
        _prog_cache[key] = _build_program(*key)
    nc = _prog_cache[key]
    in_maps = [
        {"logits8": lg8[k], "labrows": lab_rows[k], "eye": eye}
        for k in range(N_CORES)
    ]
    res = run_bass_kernel_spmd(nc, in_maps, core_ids=list(range(N_CORES)), **spmd_kwargs)
    total = np.float64(0.0)
    for r in res.results:
        total += np.float64(r["out"].sum(dtype=np.float64))
    loss = np.float32(-0.5 * total / n_tok)
    return loss, res


def kernel(logits, labels, mask_matrix):
    loss, _ = _run(logits, labels, mask_matrix)
    return loss
